# revision 25
# baseline (speedup 1.0000x reference)
"""Multi-head attention (Vaswani) on Trainium2, head-parallel across 8 NeuronCores.

Problem shapes (hardcoded):
  h:   [B=2, G=2048, D=128] f32
  W_Q/W_K/W_V: [H=8, D=128, K=16] f32
  out: [B=2, H=8, G=2048, V=16] f32  = softmax(0.25 * (h@Wq) @ (h@Wk)^T) @ (h@Wv)

Sharding: one head per core (8 heads / 8 cores). Each core receives the full h
plus its head's weight slices, computes [B, G, V]; host stacks on the head axis.

Per-core plan, all in transposed "compatT" orientation so the attention @ V
contraction lands on the partition axis with no transposes of the big G x G
attention matrix:
  1. hT[d, g] via PE transposes of [128,128] h tiles (one batched DMA per
     half-batch of h).
  2. qT[16, g], kT[16, g] = Wq^T @ hT, Wk^T @ hT (K=16 contraction).
  3. v'[m, 17] chunks = (h_chunk @ Wv | ones column); the ones column makes the
     softmax denominator accumulate in output row 16 for free.
  4. Per key chunk m (128 keys) and q-slice (1024 wide): compatT[m, q] =
     k_m . q into psum, attnT = exp(0.25 * compatT) in one wide ACT pass
     psum->sbuf, oT[17, q] += v'^T @ attnT accumulated in psum over all 16
     key chunks.  Input staging for the next batch is emitted interleaved
     into this loop so PE/DVE/DMA work fills gaps under the ACT stream
     (exp on the Scalar engine is the roofline: B*G*G/128lanes/1.2GHz
     ~ 56us per core).
  5. Transpose oT back in [17,128] blocks, scale rows by the reciprocal of
     the denominator row, one DMA per q-slice out.

The big matmul streams run as float32r (single-pass PE, ~4x fp32 throughput;
measured end-to-end rel err ~1.4e-4). Set cfg fp32r=False for full-fp32
numerics (~1.1e-6) at ~2x the runtime.
"""

import numpy as np

B, G, D = 2, 2048, 128
H, K, V = 8, 16, 16
N_CORES = 8
P = 128
GT = G // P          # 16 key/query chunks of 128
QB = 512             # one fp32 PSUM bank of free dim
NQB = G // QB        # 4
VP1 = V + 1          # v' width (ones column appended)

DEFAULT_CFG = {
    "chunk_w": 1024,   # max compat psum tile width
    "split_ends": False,  # first/last q-slices at 512
    "pc_bufs": 2,      # compat psum buffers
    "at_bufs": 6,      # attnT sbuf buffers
    "fp32r": True,     # float32r tiles for the big matmul streams
    "proj_f32": True,  # keep q/k/v projections in full fp32 (less error)
    "reps": 1,         # repeat whole kernel body (for HW slope timing)
}

_CACHE = {}


def _build(cfg_key):
    cfg = dict(DEFAULT_CFG)
    cfg.update(dict(cfg_key))
    import concourse.bacc as bacc
    import concourse.mybir as mybir
    from concourse.tile import TileContext
    from concourse.masks import make_identity

    f32 = mybir.dt.float32
    f32r = mybir.dt.float32r
    # walrus requires fp32r matmul operands to be *produced* rounded, so the
    # staging tiles themselves are declared float32r when fp32r is on.
    mdt = f32r if cfg["fp32r"] else f32
    EXP = mybir.ActivationFunctionType.Exp

    nc = bacc.Bacc("TRN2", debug=False, enable_asserts=False,
                   target_bir_lowering=False)
    h_d = nc.dram_tensor("h", [B, G, D], f32, kind="ExternalInput").ap()
    wq_d = nc.dram_tensor("wq", [D, K], f32, kind="ExternalInput").ap()
    wk_d = nc.dram_tensor("wk", [D, K], f32, kind="ExternalInput").ap()
    wv_d = nc.dram_tensor("wv", [D, V], f32, kind="ExternalInput").ap()
    out_d = nc.dram_tensor("out", [B, G, V], f32, kind="ExternalOutput").ap()

    CW = cfg["chunk_w"]
    NCW = G // CW        # q-slices per key chunk
    NQB_C = CW // QB     # matmuls (one psum bank each) per q-slice

    with TileContext(nc) as tc:
        with tc.tile_pool(name="const", bufs=1) as cpool, \
             tc.tile_pool(name="sc", bufs=2, space="PSUM") as scpool, \
             tc.tile_pool(name="pc", bufs=cfg["pc_bufs"],
                          space="PSUM") as pcpool, \
             tc.tile_pool(name="po", bufs=1, space="PSUM") as popool, \
             tc.tile_pool(name="att", bufs=cfg["at_bufs"]) as apool:
            ident = cpool.tile([P, P], f32)
            make_identity(nc, ident)
            warm = cpool.tile([P, 1], f32)
            nc.scalar.activation(warm, ident[:, 0:1], EXP)
            w_sb = cpool.tile([D, 3 * K], f32)
            pdt = f32 if cfg["proj_f32"] else mdt
            w_r = cpool.tile([D, 3 * K], pdt)

            def load_w():
                nc.sync.dma_start(w_sb[:, 0:K], wq_d)
                nc.sync.dma_start(w_sb[:, K:2 * K], wk_d)
                nc.sync.dma_start(w_sb[:, 2 * K:3 * K], wv_d)
                nc.vector.tensor_copy(w_r, w_sb)

            # Long-lived per-batch staging tiles; the zero padding and the
            # ones columns are static, so they are initialized exactly once
            # (f32r has no memset, so zeros/ones go through convert-copies).
            one = cpool.tile([P, 1], f32)
            nc.vector.memset(one, 1.0)
            hA_b, hT_b, qkT_b, kTp_b, vp_b, ob_b = [], [], [], [], [], []
            for b in range(B):
                hA_b.append(cpool.tile([P, G], f32, name=f"hA{b}"))
                hT_b.append(cpool.tile([P, G], pdt, name=f"hT{b}"))
                qkT_b.append(cpool.tile([K, G], mdt, name=f"qkT{b}"))
                kTp_b.append(cpool.tile([K, G], mdt, name=f"kTp{b}"))
                vp_b.append(cpool.tile([P, GT * VP1], mdt, name=f"vp{b}"))
                ob_b.append(cpool.tile([P, GT * V], f32, name=f"ob{b}"))

            def init_vp():
                for b in range(B):
                    for t in range(GT):
                        nc.vector.tensor_copy(
                            vp_b[b][:, t * VP1 + V:(t + 1) * VP1], one)

            def phase1_ops(b):
                """Closure list for batch b's input staging, in dependency
                order at quarter granularity; popped a few at a time inside
                the previous batch's main loop (or this batch's, for the
                trailing quarters of the first unit) so the work fills
                engine gaps while ACT streams exps."""
                hA, hT, qkT = hA_b[b], hT_b[b], qkT_b[b]
                kTp, vp = kTp_b[b], vp_b[b]

                def dmaq(qq):
                    nc.sync.dma_start(
                        hA[:, qq * 4 * P:(qq + 1) * 4 * P].rearrange(
                            "p (t d) -> p t d", t=4),
                        h_d[b, qq * 4 * P:(qq + 1) * 4 * P, :].rearrange(
                            "(t p) d -> p t d", p=P))

                def tr(t):
                    pt = scpool.tile([P, QB], f32, tag="s", name="pt")
                    nc.tensor.transpose(pt[:, 0:P],
                                        hA[:, t * P:(t + 1) * P], ident)
                    nc.vector.tensor_copy(hT[:, t * P:(t + 1) * P],
                                          pt[:, 0:P])

                def proj(qb, w0, dst):
                    sl = slice(qb * QB, (qb + 1) * QB)
                    pq = scpool.tile([P, QB], f32, tag="s", name="pq")
                    nc.tensor.matmul(pq[0:K, :], w_r[:, w0:w0 + K],
                                     hT[:, sl], start=True, stop=True)
                    nc.vector.tensor_copy(dst[:, sl], pq[0:K, :])

                def vproj(t):
                    pvv = scpool.tile([P, QB], f32, tag="s", name="pvv")
                    nc.tensor.matmul(pvv[:, 0:V], hT[:, t * P:(t + 1) * P],
                                     w_r[:, 2 * K:3 * K],
                                     start=True, stop=True)
                    nc.vector.tensor_copy(vp[:, t * VP1:t * VP1 + V],
                                          pvv[:, 0:V])

                ops = [lambda: dmaq(0), lambda: dmaq(1),
                       lambda: dmaq(2), lambda: dmaq(3)]
                for qq in range(NQB):
                    for t in range(4 * qq, 4 * qq + 4):
                        ops.append(lambda t=t: tr(t))
                    ops.append(lambda qq=qq: proj(qq, 0, qkT))
                    ops.append(lambda qq=qq: proj(qq, K, kTp))
                    for t in range(4 * qq, 4 * qq + 4):
                        ops.append(lambda t=t: vproj(t))
                # pull vprojs of the first half ahead so AV(t) is never the
                # straggler, keep projq/projk of later quarters timely
                order = [0, 1, 2, 3,            # dmas
                         4, 5, 6, 7, 8, 9,      # tr0-3 projq0 projk0
                         10, 11,                # vproj0-1
                         14, 15, 16, 17, 18, 19,  # tr4-7 projq1 projk1
                         12, 13, 20, 21,        # vproj2-3 vproj4-5
                         24, 25, 26, 27, 28, 29,  # tr8-11 projq2 projk2
                         22, 23, 30, 31,        # vproj6-7 8-9
                         34, 35, 36, 37, 38, 39,  # tr12-15 projq3 projk3
                         32, 33, 40, 41, 42, 43]  # vproj10-15
                return [ops[i] for i in order]

            units = [(rr, bb) for rr in range(cfg["reps"])
                     for bb in range(B)]
            first = phase1_ops(units[0][1])
            first = (first[0:2] + [load_w] + first[2:4] + [init_vp]
                     + first[4:])
            # prefix must cover every projection the first q-slice reads
            # (emission order IS dependency order for Tile)
            npre = 13 if cfg["split_ends"] else 20
            for op in first[:npre]:
                op()
            pending = first[npre:]
            for ui, (rep, b) in enumerate(units):
                qkT, kTp, vp, ob_all = (qkT_b[b], kTp_b[b], vp_b[b],
                                        ob_b[b])
                if ui + 1 < len(units):
                    pending = pending + phase1_ops(units[ui + 1][1])

                if cfg["split_ends"]:
                    sched = [(0, QB), (QB, CW)]
                    while sched[-1][0] + sched[-1][1] < G - QB:
                        sched.append((sched[-1][0] + sched[-1][1], CW))
                    sched.append((G - QB, QB))
                else:
                    sched = [(ci * CW, CW) for ci in range(NCW)]
                for q0, width in sched:
                    oT = popool.tile([VP1, CW], f32, tag="oT",
                                     name="oT")[:, 0:width]
                    for t in range(GT):
                        for _ in range(3):
                            if pending:
                                pending.pop(0)()
                        kT_sl = kTp[:, t * P:(t + 1) * P]
                        v_sl = vp[:, t * VP1:(t + 1) * VP1]
                        cps = pcpool.tile([P, CW], f32, tag="c",
                                          name="cps")[:, 0:width]
                        for j in range(width // QB):
                            nc.tensor.matmul(
                                cps[:, j * QB:(j + 1) * QB], kT_sl,
                                qkT[:, q0 + j * QB:q0 + (j + 1) * QB],
                                start=True, stop=True)
                        at = apool.tile([P, CW], mdt, tag="at",
                                        name="at")[:, 0:width]
                        nc.scalar.activation(at, cps, EXP, scale=0.25)
                        for j in range(width // QB):
                            nc.tensor.matmul(
                                oT[:, j * QB:(j + 1) * QB], v_sl,
                                at[:, j * QB:(j + 1) * QB],
                                start=(t == 0), stop=(t == GT - 1))

                    # normalize this q-slice: all transposes into one
                    # psum bank, then recip+scale per 128-block
                    oT_sb = apool.tile([VP1, CW], f32, tag="oTsb",
                                       name="oT_sb")[:, 0:width]
                    NT = width // P
                    half = width // 2
                    nc.vector.tensor_copy(oT_sb[:, 0:half], oT[:, 0:half])
                    nc.vector.tensor_copy(oT_sb[:, half:width],
                                          oT[:, half:width])
                    for tl in range(NT):
                        tg = (q0 + tl * P) // P
                        pf = scpool.tile([P, QB], f32, tag="s", name="pf")
                        nc.tensor.transpose(
                            pf[:, 0:VP1], oT_sb[:, tl * P:(tl + 1) * P],
                            ident[:VP1, :VP1])
                        rcp = apool.tile([P, 1], f32, tag="rcp", name="rcp")
                        nc.vector.reciprocal(rcp, pf[:, V:V + 1])
                        nc.vector.tensor_scalar_mul(
                            ob_all[:, tg * V:(tg + 1) * V], pf[:, 0:V], rcp)

                    # per-slice out DMA so the store overlaps the next
                    nc.sync.dma_start(
                        out_d[b, q0:q0 + width, :].rearrange(
                            "(t p) v -> p t v", p=P),
                        ob_all[:, (q0 // P) * V:((q0 + width) // P) * V]
                        .rearrange("p (t v) -> p t v", t=width // P))

                for op in pending:
                    op()
                pending = []

    nc.compile()
    return nc


def _get(cfg=None):
    cfg = cfg or {}
    key = tuple(sorted({**DEFAULT_CFG, **cfg}.items()))
    if key not in _CACHE:
        _CACHE[key] = _build(key)
    return _CACHE[key]


def _in_maps(h, W_Q, W_K, W_V):
    h = np.ascontiguousarray(np.asarray(h, dtype=np.float32))
    W_Q = np.asarray(W_Q, dtype=np.float32)
    W_K = np.asarray(W_K, dtype=np.float32)
    W_V = np.asarray(W_V, dtype=np.float32)
    return [
        {"h": h, "wq": np.ascontiguousarray(W_Q[c]),
         "wk": np.ascontiguousarray(W_K[c]),
         "wv": np.ascontiguousarray(W_V[c])}
        for c in range(N_CORES)
    ]


def kernel(h, W_Q, W_K, W_V, cfg=None, **run_kwargs):
    from concourse import bass_utils
    nc = _get(cfg)
    res = bass_utils.run_bass_kernel_spmd(
        nc, _in_maps(h, W_Q, W_K, W_V),
        core_ids=list(range(N_CORES)), **run_kwargs)
    out = np.stack([res.results[c]["out"] for c in range(N_CORES)], axis=1)
    kernel.last_results = res
    return out

